# revision 13
# baseline (speedup 1.0000x reference)
"""Causal self-attention (B=2, T=2048, D=1024, H=16) on 8 TRN2 NeuronCores.

Sharding: data-parallel over batch (2) x tensor-parallel over heads (4 groups
of 4 heads) = 8 cores.  Each core computes, for its (batch, head-group):
  - Q^T/K^T projections directly in [hd, T] layout (weights as lhsT, x^T as rhs)
  - V projection in natural [T, hd] layout, with a ones-column appended per
    head so the softmax denominator falls out of the P^T@V matmul for free
  - causal attention entirely in S^T = [k, q] layout (no transposes anywhere):
      S^T = K^T.T @ Q^T, additive -1e30 mask on diagonal blocks,
      P^T = exp(S^T/8) (no max-subtraction: scores are O(+-5)),
      O^T = V'.T @ P^T  (row 64 of O^T = row sums l)
      normalize O^T by 1/l broadcast across partitions
  - partial out-projection  y_part = O^T.T @ W_proj[rows of its heads]
Host sums the 4 partials per batch and adds the bias row
(b_proj + b_v @ W_proj; the K/V/proj biases commute out of the kernel:
K-bias cancels in softmax - kept anyway for fidelity - and V-bias times a
softmax row that sums to 1 becomes a constant row).

All matmuls run as float32r (full PE rate at free-dim >= 256, fp32 storage).
"""

import numpy as np

import bass_rust
import concourse.bass as bass
import concourse.mybir as mybir
import concourse.tile as tile
from concourse.bass_utils import run_bass_kernel_spmd
from concourse.vector_clock import ScopedClock

F32 = mybir.dt.float32
F32R = mybir.dt.float32r
AF = mybir.ActivationFunctionType
OP = mybir.AluOpType

B, T, D, H, HD = 2, 2048, 1024, 16, 64
NCORES = 8
HG = 4            # heads per core
GC = HG * HD      # head-group width = 256
ND = D // 128     # 8 contraction chunks
NT = T // 128     # 16 T-chunks
NTS = T // 512    # 4 T-slices / q-tiles
SCALE = 1.0 / 8.0  # 1/sqrt(HD)
NEG = -1.0e30
VW = HD + 1       # V columns per head incl. ones column


# --- workarounds for this walrus build: max ONE embedded sem-wait per inst ---

class _SplitDrainTileContext(tile.TileContext):
    """TileContext whose exit drain is split into several single-wait drains."""

    def _drain_and_barrier(self, tick_clock, wait_clock):
        drain_inst = self.nc.sync.drain()
        wait_clock.add_sem_waits(
            drain_inst.ins, ScopedClock({None: tick_clock.global_clock})
        )
        si = drain_inst.ins.sync_info
        if si is not None and len(si.on_wait) > 1:
            waits = list(si.on_wait)
            si.on_wait = waits[:1]
            drain_inst.ins.sync_info = si
            for w in waits[1:]:
                extra = self.nc.sync.drain()
                extra.ins.sync_info = bass_rust.SyncInfo(on_wait=[w], on_update=[])

        self.nc.all_engine_barrier()
        assert self.sems is not None
        popped = self.nc._tile_sem_poison_stack.pop()
        assert popped is self._sem_poison
        self.nc.clear_and_free_semaphores(list(self.sems.allocated().values()))
        self.nc.all_engine_barrier()


def _legalize_waits(nc, max_waits=1):
    """Hoist excess per-instruction sem-waits onto same-engine NoOps."""
    n_fixed = 0
    for _bb_name, bbh in list(nc.bb_map.items()):
        bb = bbh.bb if hasattr(bbh, "bb") else bbh
        insts = bb.instructions
        new_list = []
        changed = False
        for inst in insts:
            si = inst.sync_info
            if si is not None and len(si.on_wait) > max_waits:
                waits = list(si.on_wait)
                keep = waits[-max_waits:]
                extra = waits[:-max_waits]
                eng = nc.engines[inst.engine]
                for j in range(0, len(extra), max_waits):
                    nop_bi = eng.nop()
                    cur_list = nc.cur_bb.bb.instructions
                    assert cur_list[-1] is nop_bi.ins
                    cur_list.pop()
                    nop_bi.ins.sync_info = bass_rust.SyncInfo(
                        on_wait=extra[j : j + max_waits], on_update=[]
                    )
                    new_list.append(nop_bi.ins)
                si.on_wait = keep
                inst.sync_info = si
                changed = True
                n_fixed += 1
            new_list.append(inst)
        if changed:
            insts[:] = new_list
    return n_fixed


# ---------------------------- device program ----------------------------

def build_nc():

    nc = bass.Bass()
    xT = nc.declare_dram_parameter("xT", [D, T], F32R, isOutput=False)
    wqk = nc.declare_dram_parameter("wqk", [D, 2 * GC], F32R, isOutput=False)
    bqk = nc.declare_dram_parameter("bqk", [128, 4], F32, isOutput=False)
    wv = nc.declare_dram_parameter("wv", [D, GC], F32R, isOutput=False)
    wp = nc.declare_dram_parameter("wp", [GC, D], F32R, isOutput=False)
    mask = nc.declare_dram_parameter("mask", [128, 896], F32, isOutput=False)
    ones = nc.declare_dram_parameter("ones", [128, 64], F32R, isOutput=False)
    yp = nc.declare_dram_parameter("ypart", [T, D], F32, isOutput=True)

    with _SplitDrainTileContext(nc) as tc:
        with (
            tc.tile_pool(name="const", bufs=1) as const,
            tc.tile_pool(name="store", bufs=1) as store,
            tc.tile_pool(name="xstream", bufs=2) as xstream,
            tc.tile_pool(name="pt", bufs=6) as ptpool,
            tc.tile_pool(name="small", bufs=4) as small,
            tc.tile_pool(name="ysb", bufs=3) as ypool,
            tc.tile_pool(name="psum", bufs=8, space="PSUM") as psum,
        ):
            # resident weights / constants
            wqk_sb = const.tile([128, ND, 4 * 128], F32R)
            for d in range(ND):
                nc.sync.dma_start(out=wqk_sb[:, d, :], in_=wqk[d * 128:(d + 1) * 128, :])
            wv_sb = const.tile([128, ND, GC], F32R)
            for d in range(ND):
                nc.sync.dma_start(out=wv_sb[:, d, :], in_=wv[d * 128:(d + 1) * 128, :])
            wp_sb = const.tile([128, 2, D], F32R)
            for hh in range(2):
                nc.sync.dma_start(out=wp_sb[:, hh, :], in_=wp[hh * 128:(hh + 1) * 128, :])
            bqk_sb = const.tile([128, 4], F32)
            nc.sync.dma_start(out=bqk_sb[:], in_=bqk[:, :])
            mask_sb = const.tile([128, 896], F32)
            nc.sync.dma_start(out=mask_sb[:], in_=mask[:, :])

            # persistent intermediates
            qkT = store.tile([128, 4, T], F32R)       # [2*64, cc(2Q+2K), T]
            vst = store.tile([128, NT, HG * VW], F32R)  # [T-chunk rows, tc, h*65+j]
            oT = store.tile([128, 2, T], F32R)        # [2*64, head-pair, T]
            for h in range(HG):
                nc.sync.dma_start(out=vst[:, :, h * VW + HD], in_=ones[:, :NT])
            ones64 = const.tile([1, 64], F32R)
            nc.sync.dma_start(out=ones64[:], in_=ones[0:1, :])

            # ---- phase A: QKV projections ----
            for ts in range(NTS):
                xts = xstream.tile([128, ND, 512], F32R, tag="xts")
                for d in range(ND):
                    nc.sync.dma_start(
                        out=xts[:, d, :],
                        in_=xT[d * 128:(d + 1) * 128, ts * 512:(ts + 1) * 512],
                    )
                # Q^T / K^T : out [col-chunk(128), 512 q's]
                for cc in range(4):
                    ps = psum.tile([128, 512], F32, tag="ps")
                    for d in range(ND):
                        nc.tensor.matmul(
                            ps[:],
                            lhsT=(wqk_sb[:, d, cc * 128:(cc + 1) * 128]),
                            rhs=(xts[:, d, :]),
                            start=(d == 0),
                            stop=(d == ND - 1),
                        )
                    nc.scalar.activation(
                        qkT[:, cc, ts * 512:(ts + 1) * 512], ps[:],
                        AF.Identity, bias=bqk_sb[:, cc:cc + 1],
                    )
                # V : out [T-chunk(128), 256]
                for ti in range(4):
                    tchunk = ts * 4 + ti
                    psv = psum.tile([128, 512], F32, tag="ps")
                    for d in range(ND):
                        nc.tensor.matmul(
                            psv[:, :GC],
                            lhsT=(xts[:, d, ti * 128:(ti + 1) * 128]),
                            rhs=(wv_sb[:, d, :]),
                            start=(d == 0),
                            stop=(d == ND - 1),
                        )
                    for h in range(HG):
                        nc.vector.tensor_copy(
                            vst[:, tchunk, h * VW:h * VW + HD],
                            psv[:, h * HD:(h + 1) * HD],
                        )

            # ---- phase B: attention, S^T layout ----
            for h in range(HG):
                po = (h % 2) * 64          # partition offset of this head
                ccq, cck = h // 2, 2 + h // 2
                for qt in range(NTS):
                    nkc = 4 * qt + 4       # causal: k-chunks 0 .. 4*qt+3
                    pv = psum.tile([128, 512], F32, tag="ps")

                    def st_step(kc):
                        st = psum.tile([128, 512], F32, tag="ps")
                        nc.tensor.matmul(
                            st[:],
                            lhsT=(qkT[po:po + HD, cck, kc * 128:(kc + 1) * 128]),
                            rhs=(qkT[po:po + HD, ccq, qt * 512:(qt + 1) * 512]),
                            start=True, stop=True,
                        )
                        dd = (kc - 4 * qt) * 128
                        if dd >= 0:  # diagonal block: causal mask
                            nc.vector.tensor_tensor(
                                out=st[:], in0=st[:],
                                in1=mask_sb[:, 384 - dd:896 - dd], op=OP.add,
                            )
                        pt = ptpool.tile([128, 512], F32R, tag="pt")
                        nc.scalar.activation(pt[:], st[:], AF.Exp, scale=SCALE)
                        return pt

                    def pv_step(kc, pt):
                        nc.tensor.matmul(
                            pv[:VW, :],
                            lhsT=(vst[:, kc, h * VW:(h + 1) * VW]),
                            rhs=(pt[:]),
                            start=(kc == 0),
                            stop=(kc == nkc - 1),
                            skip_group_check=True,
                        )

                    prev = st_step(0)
                    for kc in range(1, nkc):
                        cur = st_step(kc)
                        pv_step(kc - 1, prev)
                        prev = cur
                    pv_step(nkc - 1, prev)

                    # normalize: O^T[hd, q] *= 1/l[q]
                    # (1/l broadcast across partitions via rank-1 PE matmul)
                    lr = small.tile([1, 512], F32R, tag="lr")
                    with nc.allow_low_precision(reason="fp32r rounding of 1/l"):
                        nc.vector.reciprocal(lr[:], pv[HD:HD + 1, :])
                    bps = psum.tile([128, 512], F32, tag="ps")
                    nc.tensor.matmul(
                        bps[:HD, :], lhsT=ones64[:], rhs=lr[:],
                        start=True, stop=True,
                    )
                    rb = small.tile([64, 512], F32, tag="rb")
                    nc.vector.tensor_copy(rb[:], bps[:HD, :])
                    nc.vector.tensor_tensor(
                        out=oT[po:po + HD, h // 2, qt * 512:(qt + 1) * 512],
                        in0=pv[:HD, :], in1=rb[:], op=OP.mult,
                    )

            # ---- phase C: out-projection partials ----
            for tchunk in range(NT):
                for half in range(2):
                    py = psum.tile([128, 512], F32, tag="ps")
                    for hh in range(2):
                        nc.tensor.matmul(
                            py[:],
                            lhsT=(oT[:, hh, tchunk * 128:(tchunk + 1) * 128]),
                            rhs=(wp_sb[:, hh, half * 512:(half + 1) * 512]),
                            start=(hh == 0),
                            stop=(hh == 1),
                        )
                    ys = ypool.tile([128, 512], F32, tag="ys")
                    nc.scalar.activation(ys[:], py[:], AF.Copy)
                    nc.sync.dma_start(
                        out=yp[tchunk * 128:(tchunk + 1) * 128,
                               half * 512:(half + 1) * 512],
                        in_=ys[:],
                    )

    _legalize_waits(nc)
    return nc


_NC = None


def _get_nc():
    global _NC
    if _NC is None:
        _NC = build_nc()
    return _NC


def _causal_neg_mask():
    # master mask M[k, c] = 0 if c >= k + 384 else NEG; view for diagonal
    # offset dd is M[:, 384-dd : 896-dd] giving 0 where q >= k + dd.
    k = np.arange(128)[:, None]
    c = np.arange(896)[None, :]
    return np.where(c >= k + 384, 0.0, NEG).astype(np.float32)


def make_in_maps(x, W_qkv, b_qkv, W_proj):
    mask = _causal_neg_mask()
    xTs = [np.ascontiguousarray(x[b].T) for b in range(B)]
    in_maps = []
    for c in range(NCORES):
        b, g = divmod(c, HG)
        cols_q = slice(g * GC, (g + 1) * GC)
        cols_k = slice(D + g * GC, D + (g + 1) * GC)
        cols_v = slice(2 * D + g * GC, 2 * D + (g + 1) * GC)
        wqk = np.ascontiguousarray(
            np.concatenate([W_qkv[:, cols_q], W_qkv[:, cols_k]], axis=1))
        bqk = np.ascontiguousarray(
            np.concatenate([b_qkv[cols_q], b_qkv[cols_k]]).reshape(4, 128).T)
        wv = np.ascontiguousarray(W_qkv[:, cols_v])
        wp = np.ascontiguousarray(W_proj[g * GC:(g + 1) * GC, :])
        in_maps.append({
            "xT": xTs[b], "wqk": wqk, "bqk": bqk, "wv": wv, "wp": wp,
            "mask": mask, "ones": np.ones((128, 64), np.float32),
        })
    return in_maps


def kernel(x, W_qkv, b_qkv, W_proj, b_proj):
    x = np.asarray(x, np.float32)
    W_qkv = np.asarray(W_qkv, np.float32)
    b_qkv = np.asarray(b_qkv, np.float32)
    W_proj = np.asarray(W_proj, np.float32)
    b_proj = np.asarray(b_proj, np.float32)

    nc = _get_nc()
    in_maps = make_in_maps(x, W_qkv, b_qkv, W_proj)
    res = run_bass_kernel_spmd(nc, in_maps, list(range(NCORES)))

    # host-side gather: sum head-group partials per batch + bias row
    bias_row = b_proj + b_qkv[2 * D:].astype(np.float32) @ W_proj
    y = np.empty((B, T, D), np.float32)
    for b in range(B):
        acc = res.results[4 * b]["ypart"].astype(np.float32).copy()
        for g in range(1, HG):
            acc += res.results[4 * b + g]["ypart"]
        y[b] = acc + bias_row
    return y
